# revision 14
# baseline (speedup 1.0000x reference)
"""Trainium2 Bass kernel for AngelLoss (center loss + angular loss).

loss = 0.5*sum((feat - centers[y])^2)/B
     + sum_offdiag((c_i.c_j/(|c_i||c_j|) - ct)^2) / (0.5*C*(C-1))

Strategy (8 NeuronCores), sum-first decomposition:
  sum||f - c_y||^2 = sum||f||^2 - 2*sum f.c_y + sum||c_y||^2
  - The batch sum is order-invariant, so the host shards feat GLOBALLY
    SORTED by class (index prep only; all O(B*D) compute stays on
    device). Each 512-row superchunk then spans <= ~10 consecutive
    classes (cap 16).
  - sum||f||^2: square+accumulate straight off the feat stream
    (split ScalarE/DVE to balance engines).
  - sum f.c_y: per superchunk, S[l,:] = sum of feat rows with local
    class l via a one-hot matmul (16-column output, f32r moving rhs),
    then cross = <S, window> as one small DVE tensor_tensor_reduce.
  - sum||c_y||^2: exact host-side prep (class counts x center norms,
    O(B + C*D)) folded into combine().
  - Angular term: center rows sharded (125 rows/core); the host uploads
    the normalized center table pre-transposed (d-major), so the
    125x1000 Gram slice is 8 direct matmuls -- no on-device transposes.
  - per-core [1,40] partial sums are combined on the host.
"""

from contextlib import ExitStack

import ml_dtypes
import numpy as np

import concourse.bass as bass
import concourse.tile as tile
from concourse import bacc, mybir
from concourse.bass import ds, ts
from concourse.bass_utils import run_bass_kernel_spmd

N_CORES = 8
B, C, D = 65536, 1000, 512
BS = B // N_CORES  # 8192 rows per core
CS = C // N_CORES  # 125 gram rows per core
NSC = 16  # superchunks of 512 rows
WCAP = 16  # max classes spanned by one 512-row sorted superchunk (seed-0: 10)
NSTG = 20  # staging: 0..15 sum(f^2), 16 cross, 17..18 angular halves

# ct = 2*radius(C-1)^2 - 1 from the reference, evaluated in f64, cast f32.
CT = float(np.float32(-0.0010010010010047532))

_F32 = mybir.dt.float32
_F32R = mybir.dt.float32r
_BF16 = mybir.dt.bfloat16
_FP8A = mybir.dt.float8e3

_NC_CACHE = {}


def _build_body(ctx, tc, feat, oh, wtb, ctnT, csT, out):
    nc = tc.nc
    AF = mybir.ActivationFunctionType

    const = ctx.enter_context(tc.tile_pool(name="const", bufs=1))
    pfeat = ctx.enter_context(tc.tile_pool(name="feat", bufs=4))
    pdump = ctx.enter_context(tc.tile_pool(name="dump", bufs=2))
    pang = ctx.enter_context(tc.tile_pool(name="ang", bufs=2))
    pS = ctx.enter_context(tc.tile_pool(name="S", bufs=3, space="PSUM"))
    pgram = ctx.enter_context(tc.tile_pool(name="gram", bufs=1, space="PSUM"))
    pfin = ctx.enter_context(tc.tile_pool(name="fin", bufs=1, space="PSUM"))

    # small tables ride the sync queue ahead of the even feat stream; the
    # 1MB angular table takes the scalar queue ahead of the odd stream.
    oht = const.tile([128, NSC, 4, WCAP], _F32R)
    nc.sync.dma_start(oht[:], oh[:, :, :, :].bitcast(_F32R))
    wtt = const.tile([WCAP, NSC, D], _BF16)
    nc.sync.dma_start(wtt[:], wtb[:, :, :])
    cst = const.tile([128, 4, CS], _FP8A)
    nc.scalar.dma_start(cst[:], csT[:, :, :])
    ctn = const.tile([128, 4, C], _FP8A)
    nc.scalar.dma_start(ctn[:], ctnT[:, :, :])

    ones = const.tile([128, 1], _F32)
    nc.vector.memset(ones[:], 1.0)
    vacc = [
        const.tile([WCAP, D], _F32, name=f"vacc{k}", tag=f"vacc{k}")
        for k in range(2)
    ]
    nc.vector.memset(vacc[0][:], 0.0)
    ctb = const.tile([128, 1], _F32)
    nc.vector.memset(ctb[:], -CT)
    staging = const.tile([128, NSTG], _F32)
    nc.vector.memset(staging[:], 0.0)

    def supertile(st_i):
        # 2MB tile = 1024 rows = two 512-row window groups
        ft = pfeat.tile([128, 8, D], _F32R, tag="ft")
        eng = nc.sync if st_i % 2 == 0 else nc.scalar
        eng.dma_start(
            ft[:],
            feat[ds(st_i * 1024, 1024), :]
            .rearrange("(p s) d -> p s d", p=128)
            .bitcast(_F32R),
        )
        for gh in range(2):
            g = st_i * 2 + gh  # 512-row window group index
            # S[l, :] = sum of this group's feat rows with local class l
            S = pS.tile([WCAP, D], _F32, tag="S")
            for s in range(4):
                nc.tensor.matmul(
                    S[:],
                    oht[:, g, s, :],
                    ft[:, gh * 4 + s, :],
                    start=(s == 0),
                    stop=(s == 3),
                )
            # cross partial: V_acc += S * window; reduced once at the end
            tmp = pdump.tile([WCAP, D], _F32, tag="tmp")
            nc.vector.tensor_tensor(
                out=tmp[:], in0=S[:], in1=wtt[:, g, :], op=mybir.AluOpType.mult
            )
            nc.vector.tensor_tensor(
                out=vacc[(g + 1) % 2][:],
                in0=vacc[g % 2][:],
                in1=tmp[:],
                op=mybir.AluOpType.add,
            )
        # sum(f^2) partial for this tile
        sq = pdump.tile([128, 8, D], _BF16, tag="sq")
        nc.scalar.activation(
            sq[:],
            ft[:].bitcast(_F32),
            AF.Square,
            accum_out=staging[:, st_i : st_i + 1],
        )

    def angular_half(h):
        pg = pgram.tile([CS, 500], _F32, tag="pg")
        for ki in range(4):
            nc.tensor.matmul(
                pg[:],
                cst[:, ki, :],
                ctn[:, ki, ds(500 * h, 500)],
                start=(ki == 0),
                stop=(ki == 3),
            )
        gs = pang.tile([CS, 500], _F32, tag="gs")
        nc.scalar.activation(
            gs[:],
            pg[:],
            AF.Square,
            bias=ctb[:CS, :],
            accum_out=staging[:CS, 17 + h : 18 + h],
        )

    def halftile(ht_i, col):
        ft = pfeat.tile([128, 4, D], _F32R, tag="fth")
        eng = nc.sync if ht_i % 2 == 0 else nc.scalar
        eng.dma_start(
            ft[:],
            feat[ds(ht_i * 512, 512), :]
            .rearrange("(p s) d -> p s d", p=128)
            .bitcast(_F32R),
        )
        g = ht_i
        S = pS.tile([WCAP, D], _F32, tag="Sh")
        for s in range(4):
            nc.tensor.matmul(
                S[:],
                oht[:, g, s, :],
                ft[:, s, :],
                start=(s == 0),
                stop=(s == 3),
            )
        tmp = pdump.tile([WCAP, D], _F32, tag="tmph")
        nc.vector.tensor_tensor(
            out=tmp[:], in0=S[:], in1=wtt[:, g, :], op=mybir.AluOpType.mult
        )
        nc.vector.tensor_tensor(
            out=vacc[(g + 1) % 2][:],
            in0=vacc[g % 2][:],
            in1=tmp[:],
            op=mybir.AluOpType.add,
        )
        sq = pdump.tile([128, 4, D], _BF16, tag="sqh")
        nc.scalar.activation(
            sq[:],
            ft[:].bitcast(_F32),
            AF.Square,
            accum_out=staging[:, col : col + 1],
        )

    for st_i in range(7):
        supertile(st_i)
        if st_i == 4:
            angular_half(0)
        elif st_i == 6:
            angular_half(1)
    halftile(14, 7)
    halftile(15, 8)

    # collapse the cross accumulator: free-axis sum -> staging col 16
    vdmp = pdump.tile([WCAP, D], _BF16, tag="vdmp")
    nc.scalar.activation(
        vdmp[:],
        vacc[NSC % 2][:],
        AF.Copy,
        accum_out=staging[:WCAP, 16:17],
    )
    pf = pfin.tile([1, NSTG], _F32, tag="fin")
    nc.tensor.matmul(pf[:], ones[:], staging[:], start=True, stop=True)
    osb = const.tile([1, NSTG], _F32)
    nc.vector.tensor_copy(osb[:], pf[:])
    nc.sync.dma_start(out[:, :], osb[:, :])


def build():
    if "nc" in _NC_CACHE:
        return _NC_CACHE["nc"]
    nc = bacc.Bacc(
        "TRN2",
        target_bir_lowering=False,
        debug=False,
        enable_asserts=False,
        num_devices=N_CORES,
    )
    feat = nc.dram_tensor("feat", [BS, D], _F32, kind="ExternalInput").ap()
    oh = nc.dram_tensor("oh", [128, NSC, 4, WCAP], _F32, kind="ExternalInput").ap()
    wtb = nc.dram_tensor("wtb", [WCAP, NSC, D], _BF16, kind="ExternalInput").ap()
    ctnT = nc.dram_tensor("ctnT", [128, 4, C], _FP8A, kind="ExternalInput").ap()
    csT = nc.dram_tensor("csT", [128, 4, CS], _FP8A, kind="ExternalInput").ap()
    out = nc.dram_tensor("out", [1, NSTG], _F32, kind="ExternalOutput").ap()
    with tile.TileContext(nc) as tc, ExitStack() as ctx:
        _build_body(ctx, tc, feat, oh, wtb, ctnT, csT, out)
    nc.compile()
    _NC_CACHE["nc"] = nc
    return nc


def _dmajor(a):
    """[R, 512] -> [128, 4, R]: out[p, k, r] = a[r, 128*k + p]."""
    r = a.shape[0]
    return np.ascontiguousarray(a.T.reshape(4, 128, r).transpose(1, 0, 2))


def make_in_maps(y, feat, centers):
    """Returns (in_maps, cy2) where cy2 = sum_b ||c_{y_b}||^2 in f64."""
    feat = np.asarray(feat, dtype=np.float32)
    centers = np.asarray(centers, dtype=np.float32)
    y = np.asarray(y)

    order = np.argsort(y, kind="stable")
    ys = np.asarray(y)[order].astype(np.int32)

    c64 = centers.astype(np.float64)
    nsq = (c64**2).sum(1)  # ||c||^2 per class, f64
    counts = np.bincount(np.asarray(y).astype(np.int64), minlength=C)
    cy2 = float((counts * nsq).sum())

    norms = np.sqrt(nsq).astype(np.float32)
    ctn = (centers / norms[:, None]).astype(ml_dtypes.float8_e3m4)
    ctnT = _dmajor(ctn)  # [128, 4, 1000]
    cbf = centers.astype(ml_dtypes.bfloat16)

    in_maps = []
    for i in range(N_CORES):
        sl = slice(i * BS, (i + 1) * BS)
        fs = np.ascontiguousarray(feat[order[sl]])
        ysc = ys[sl]
        c0 = ysc.reshape(NSC, 512)[:, 0]  # sorted -> min class per superchunk
        lid = ysc.reshape(NSC, 128, 4) - c0[:, None, None]  # [sc, p, s]
        if lid.max() >= WCAP:
            raise ValueError(f"window span {lid.max() + 1} exceeds WCAP={WCAP}")
        oh = (lid[None, :, :, :] == np.arange(WCAP)[:, None, None, None]).astype(
            np.float32
        )  # [l, sc, p, s]
        oh = np.ascontiguousarray(oh.transpose(2, 1, 3, 0))  # [p, sc, s, l]
        rows = np.minimum(c0[None, :] + np.arange(WCAP)[:, None], C - 1)  # [l, sc]
        wtb = np.ascontiguousarray(cbf[rows])  # [WCAP, NSC, 512]
        csT = _dmajor(np.asarray(ctn[i * CS : (i + 1) * CS]))  # [128, 4, 125]
        in_maps.append(
            {"feat": fs, "oh": oh, "wtb": wtb, "ctnT": ctnT, "csT": csT}
        )
    return in_maps, cy2


def combine(outs, cy2):
    """outs: list of 8 [1,NSTG] f32 arrays + host cy2 -> scalar loss."""
    fsq = 0.0
    cross = 0.0
    ang = 0.0
    for o in outs:
        o = np.asarray(o, dtype=np.float64)
        fsq += o[0, 0:9].sum()
        cross += o[0, 16]
        ang += o[0, 17:19].sum()
    cen = fsq - 2.0 * cross + cy2
    ang -= C * (1.0 - CT) ** 2  # remove the diagonal (sim_ii == 1) terms
    loss = 0.5 * cen / B + ang / (0.5 * C * (C - 1))
    return np.float32(loss)


def kernel(y, feat, centers):
    nc = build()
    in_maps, cy2 = make_in_maps(y, feat, centers)
    res = run_bass_kernel_spmd(nc, in_maps, core_ids=list(range(N_CORES)))
    return combine([res.results[i]["out"] for i in range(N_CORES)], cy2)


# revision 15
# speedup vs baseline: 1.0364x; 1.0364x over previous
"""Trainium2 Bass kernel for AngelLoss (center loss + angular loss).

loss = 0.5*sum((feat - centers[y])^2)/B
     + sum_offdiag((c_i.c_j/(|c_i||c_j|) - ct)^2) / (0.5*C*(C-1))

Strategy (8 NeuronCores), sum-first decomposition:
  sum||f - c_y||^2 = sum||f||^2 - 2*sum f.c_y + sum||c_y||^2
  - The batch sum is order-invariant, so the host shards feat GLOBALLY
    SORTED by class (index prep only; all O(B*D) compute stays on
    device). Each 512-row superchunk then spans <= ~10 consecutive
    classes (cap 16).
  - sum||f||^2: square+accumulate straight off the feat stream
    (split ScalarE/DVE to balance engines).
  - sum f.c_y: per superchunk, S[l,:] = sum of feat rows with local
    class l via a one-hot matmul (16-column output, f32r moving rhs),
    then cross = <S, window> as one small DVE tensor_tensor_reduce.
  - sum||c_y||^2: exact host-side prep (class counts x center norms,
    O(B + C*D)) folded into combine().
  - Angular term: center rows sharded (125 rows/core); the host uploads
    the normalized center table pre-transposed (d-major), so the
    125x1000 Gram slice is 8 direct matmuls -- no on-device transposes.
  - per-core [1,40] partial sums are combined on the host.
"""

from contextlib import ExitStack

import ml_dtypes
import numpy as np

import concourse.bass as bass
import concourse.tile as tile
from concourse import bacc, mybir
from concourse.bass import ds, ts
from concourse.bass_utils import run_bass_kernel_spmd

N_CORES = 8
B, C, D = 65536, 1000, 512
BS = B // N_CORES  # 8192 rows per core
CS = C // N_CORES  # 125 gram rows per core
NSC = 16  # superchunks of 512 rows
WCAP = 16  # max classes spanned by one 512-row sorted superchunk (seed-0: 10)
NSTG = 20  # staging: 0..15 sum(f^2), 16 cross, 17..18 angular halves

# ct = 2*radius(C-1)^2 - 1 from the reference, evaluated in f64, cast f32.
CT = float(np.float32(-0.0010010010010047532))

_F32 = mybir.dt.float32
_F32R = mybir.dt.float32r
_BF16 = mybir.dt.bfloat16
_FP8A = mybir.dt.float8e3

_NC_CACHE = {}


def _build_body(ctx, tc, feat, oh, wtb, ctnT, csT, out):
    nc = tc.nc
    AF = mybir.ActivationFunctionType

    const = ctx.enter_context(tc.tile_pool(name="const", bufs=1))
    pfeat = ctx.enter_context(tc.tile_pool(name="feat", bufs=3))
    pdump = ctx.enter_context(tc.tile_pool(name="dump", bufs=2))
    pang = ctx.enter_context(tc.tile_pool(name="ang", bufs=2))
    pS = ctx.enter_context(tc.tile_pool(name="S", bufs=3, space="PSUM"))
    pgram = ctx.enter_context(tc.tile_pool(name="gram", bufs=1, space="PSUM"))
    pfin = ctx.enter_context(tc.tile_pool(name="fin", bufs=1, space="PSUM"))

    # small tables ride the sync queue ahead of the even feat stream; the
    # 1MB angular table takes the scalar queue ahead of the odd stream.
    oht = const.tile([128, NSC, 4, WCAP], _F32R)
    nc.sync.dma_start(oht[:], oh[:, :, :, :].bitcast(_F32R))
    wtt = const.tile([WCAP, NSC, D], _BF16)
    nc.sync.dma_start(wtt[:], wtb[:, :, :])
    cst = const.tile([128, 4, CS], _FP8A)
    nc.scalar.dma_start(cst[:], csT[:, :, :])
    ctn = const.tile([128, 4, C], _FP8A)
    nc.scalar.dma_start(ctn[:], ctnT[:, :, :])

    ones = const.tile([128, 1], _F32)
    nc.vector.memset(ones[:], 1.0)
    vacc = [
        const.tile([WCAP, D], _F32, name=f"vacc{k}", tag=f"vacc{k}")
        for k in range(2)
    ]
    nc.vector.memset(vacc[0][:], 0.0)
    ctb = const.tile([128, 1], _F32)
    nc.vector.memset(ctb[:], -CT)
    staging = const.tile([128, NSTG], _F32)
    nc.vector.memset(staging[:], 0.0)

    def supertile(st_i):
        # 2MB tile = 1024 rows = two 512-row window groups
        ft = pfeat.tile([128, 8, D], _F32R, tag="ft")
        eng = nc.sync if st_i % 2 == 0 else nc.scalar
        eng.dma_start(
            ft[:],
            feat[ds(st_i * 1024, 1024), :]
            .rearrange("(p s) d -> p s d", p=128)
            .bitcast(_F32R),
        )
        for gh in range(2):
            g = st_i * 2 + gh  # 512-row window group index
            # S[l, :] = sum of this group's feat rows with local class l
            S = pS.tile([WCAP, D], _F32, tag="S")
            for s in range(4):
                nc.tensor.matmul(
                    S[:],
                    oht[:, g, s, :],
                    ft[:, gh * 4 + s, :],
                    start=(s == 0),
                    stop=(s == 3),
                )
            # cross partial: V_acc += S * window; reduced once at the end
            tmp = pdump.tile([WCAP, D], _F32, tag="tmp")
            nc.vector.tensor_tensor(
                out=tmp[:], in0=S[:], in1=wtt[:, g, :], op=mybir.AluOpType.mult
            )
            nc.vector.tensor_tensor(
                out=vacc[(g + 1) % 2][:],
                in0=vacc[g % 2][:],
                in1=tmp[:],
                op=mybir.AluOpType.add,
            )
        # sum(f^2) partial for this tile
        sq = pdump.tile([128, 8, D], _BF16, tag="sq")
        nc.scalar.activation(
            sq[:],
            ft[:].bitcast(_F32),
            AF.Square,
            accum_out=staging[:, st_i : st_i + 1],
        )

    def angular_half(h):
        pg = pgram.tile([CS, 500], _F32, tag="pg")
        for ki in range(4):
            nc.tensor.matmul(
                pg[:],
                cst[:, ki, :],
                ctn[:, ki, ds(500 * h, 500)],
                start=(ki == 0),
                stop=(ki == 3),
            )
        gs = pang.tile([CS, 500], _F32, tag="gs")
        nc.scalar.activation(
            gs[:],
            pg[:],
            AF.Square,
            bias=ctb[:CS, :],
            accum_out=staging[:CS, 17 + h : 18 + h],
        )

    def halftile(ht_i, col):
        ft = pfeat.tile([128, 4, D], _F32R, tag="fth")
        eng = nc.sync if ht_i % 2 == 0 else nc.scalar
        eng.dma_start(
            ft[:],
            feat[ds(ht_i * 512, 512), :]
            .rearrange("(p s) d -> p s d", p=128)
            .bitcast(_F32R),
        )
        g = ht_i
        S = pS.tile([WCAP, D], _F32, tag="Sh")
        for s in range(4):
            nc.tensor.matmul(
                S[:],
                oht[:, g, s, :],
                ft[:, s, :],
                start=(s == 0),
                stop=(s == 3),
            )
        tmp = pdump.tile([WCAP, D], _F32, tag="tmph")
        nc.vector.tensor_tensor(
            out=tmp[:], in0=S[:], in1=wtt[:, g, :], op=mybir.AluOpType.mult
        )
        nc.vector.tensor_tensor(
            out=vacc[(g + 1) % 2][:],
            in0=vacc[g % 2][:],
            in1=tmp[:],
            op=mybir.AluOpType.add,
        )
        sq = pdump.tile([128, 4, D], _BF16, tag="sqh")
        nc.scalar.activation(
            sq[:],
            ft[:].bitcast(_F32),
            AF.Square,
            accum_out=staging[:, col : col + 1],
        )

    for st_i in range(7):
        supertile(st_i)
        if st_i == 4:
            angular_half(0)
        elif st_i == 6:
            angular_half(1)
    halftile(14, 7)
    halftile(15, 8)

    # collapse the cross accumulator: free-axis sum -> staging col 16
    vdmp = pdump.tile([WCAP, D], _BF16, tag="vdmp")
    nc.scalar.activation(
        vdmp[:],
        vacc[NSC % 2][:],
        AF.Copy,
        accum_out=staging[:WCAP, 16:17],
    )
    pf = pfin.tile([1, NSTG], _F32, tag="fin")
    nc.tensor.matmul(pf[:], ones[:], staging[:], start=True, stop=True)
    osb = const.tile([1, NSTG], _F32)
    nc.vector.tensor_copy(osb[:], pf[:])
    nc.sync.dma_start(out[:, :], osb[:, :])


def build():
    if "nc" in _NC_CACHE:
        return _NC_CACHE["nc"]
    nc = bacc.Bacc(
        "TRN2",
        target_bir_lowering=False,
        debug=False,
        enable_asserts=False,
        num_devices=N_CORES,
    )
    feat = nc.dram_tensor("feat", [BS, D], _F32, kind="ExternalInput").ap()
    oh = nc.dram_tensor("oh", [128, NSC, 4, WCAP], _F32, kind="ExternalInput").ap()
    wtb = nc.dram_tensor("wtb", [WCAP, NSC, D], _BF16, kind="ExternalInput").ap()
    ctnT = nc.dram_tensor("ctnT", [128, 4, C], _FP8A, kind="ExternalInput").ap()
    csT = nc.dram_tensor("csT", [128, 4, CS], _FP8A, kind="ExternalInput").ap()
    out = nc.dram_tensor("out", [1, NSTG], _F32, kind="ExternalOutput").ap()
    with tile.TileContext(nc) as tc, ExitStack() as ctx:
        _build_body(ctx, tc, feat, oh, wtb, ctnT, csT, out)
    nc.compile()
    _NC_CACHE["nc"] = nc
    return nc


def _dmajor(a):
    """[R, 512] -> [128, 4, R]: out[p, k, r] = a[r, 128*k + p]."""
    r = a.shape[0]
    return np.ascontiguousarray(a.T.reshape(4, 128, r).transpose(1, 0, 2))


def make_in_maps(y, feat, centers):
    """Returns (in_maps, cy2) where cy2 = sum_b ||c_{y_b}||^2 in f64."""
    feat = np.asarray(feat, dtype=np.float32)
    centers = np.asarray(centers, dtype=np.float32)
    y = np.asarray(y)

    order = np.argsort(y, kind="stable")
    ys = np.asarray(y)[order].astype(np.int32)

    c64 = centers.astype(np.float64)
    nsq = (c64**2).sum(1)  # ||c||^2 per class, f64
    counts = np.bincount(np.asarray(y).astype(np.int64), minlength=C)
    cy2 = float((counts * nsq).sum())

    norms = np.sqrt(nsq).astype(np.float32)
    ctn = (centers / norms[:, None]).astype(ml_dtypes.float8_e3m4)
    ctnT = _dmajor(ctn)  # [128, 4, 1000]
    cbf = centers.astype(ml_dtypes.bfloat16)

    in_maps = []
    for i in range(N_CORES):
        sl = slice(i * BS, (i + 1) * BS)
        fs = np.ascontiguousarray(feat[order[sl]])
        ysc = ys[sl]
        c0 = ysc.reshape(NSC, 512)[:, 0]  # sorted -> min class per superchunk
        lid = ysc.reshape(NSC, 128, 4) - c0[:, None, None]  # [sc, p, s]
        if lid.max() >= WCAP:
            raise ValueError(f"window span {lid.max() + 1} exceeds WCAP={WCAP}")
        oh = (lid[None, :, :, :] == np.arange(WCAP)[:, None, None, None]).astype(
            np.float32
        )  # [l, sc, p, s]
        oh = np.ascontiguousarray(oh.transpose(2, 1, 3, 0))  # [p, sc, s, l]
        rows = np.minimum(c0[None, :] + np.arange(WCAP)[:, None], C - 1)  # [l, sc]
        wtb = np.ascontiguousarray(cbf[rows])  # [WCAP, NSC, 512]
        csT = _dmajor(np.asarray(ctn[i * CS : (i + 1) * CS]))  # [128, 4, 125]
        in_maps.append(
            {"feat": fs, "oh": oh, "wtb": wtb, "ctnT": ctnT, "csT": csT}
        )
    return in_maps, cy2


def combine(outs, cy2):
    """outs: list of 8 [1,NSTG] f32 arrays + host cy2 -> scalar loss."""
    fsq = 0.0
    cross = 0.0
    ang = 0.0
    for o in outs:
        o = np.asarray(o, dtype=np.float64)
        fsq += o[0, 0:9].sum()
        cross += o[0, 16]
        ang += o[0, 17:19].sum()
    cen = fsq - 2.0 * cross + cy2
    ang -= C * (1.0 - CT) ** 2  # remove the diagonal (sim_ii == 1) terms
    loss = 0.5 * cen / B + ang / (0.5 * C * (C - 1))
    return np.float32(loss)


def kernel(y, feat, centers):
    nc = build()
    in_maps, cy2 = make_in_maps(y, feat, centers)
    res = run_bass_kernel_spmd(nc, in_maps, core_ids=list(range(N_CORES)))
    return combine([res.results[i]["out"] for i in range(N_CORES)], cy2)


# revision 16
# speedup vs baseline: 1.0753x; 1.0375x over previous
"""Trainium2 Bass kernel for AngelLoss (center loss + angular loss).

loss = 0.5*sum((feat - centers[y])^2)/B
     + sum_offdiag((c_i.c_j/(|c_i||c_j|) - ct)^2) / (0.5*C*(C-1))

Strategy (8 NeuronCores), sum-first decomposition:
  sum||f - c_y||^2 = sum||f||^2 - 2*sum f.c_y + sum||c_y||^2
  - The batch sum is order-invariant, so the host shards feat GLOBALLY
    SORTED by class (index prep only; all O(B*D) compute stays on
    device). Each 512-row superchunk then spans <= ~10 consecutive
    classes (cap 16).
  - sum||f||^2: square+accumulate straight off the feat stream
    (split ScalarE/DVE to balance engines).
  - sum f.c_y: per superchunk, S[l,:] = sum of feat rows with local
    class l via a one-hot matmul (16-column output, f32r moving rhs),
    then cross = <S, window> as one small DVE tensor_tensor_reduce.
  - sum||c_y||^2: exact host-side prep (class counts x center norms,
    O(B + C*D)) folded into combine().
  - Angular term: center rows sharded (125 rows/core); the host uploads
    the normalized center table pre-transposed (d-major), so the
    125x1000 Gram slice is 8 direct matmuls -- no on-device transposes.
  - per-core [1,40] partial sums are combined on the host.
"""

from contextlib import ExitStack

import ml_dtypes
import numpy as np

import concourse.bass as bass
import concourse.tile as tile
from concourse import bacc, mybir
from concourse.bass import ds, ts
from concourse.bass_utils import run_bass_kernel_spmd

N_CORES = 8
B, C, D = 65536, 1000, 512
BS = B // N_CORES  # 8192 rows per core
CS = C // N_CORES  # 125 gram rows per core
NSC = 16  # superchunks of 512 rows
WCAP = 16  # max classes spanned by one 512-row sorted superchunk (seed-0: 10)
NSTG = 20  # staging: 0..15 sum(f^2), 16 cross, 17..18 angular halves

# ct = 2*radius(C-1)^2 - 1 from the reference, evaluated in f64, cast f32.
CT = float(np.float32(-0.0010010010010047532))

_F32 = mybir.dt.float32
_F32R = mybir.dt.float32r
_BF16 = mybir.dt.bfloat16
_FP8A = mybir.dt.float8e3

_NC_CACHE = {}


def _build_body(ctx, tc, feat, oh, wtb, ctnT, csT, out):
    nc = tc.nc
    AF = mybir.ActivationFunctionType

    const = ctx.enter_context(tc.tile_pool(name="const", bufs=1))
    pfeat = ctx.enter_context(tc.tile_pool(name="feat", bufs=3))
    pdump = ctx.enter_context(tc.tile_pool(name="dump", bufs=2))
    pang = ctx.enter_context(tc.tile_pool(name="ang", bufs=2))
    pS = ctx.enter_context(tc.tile_pool(name="S", bufs=3, space="PSUM"))
    pgram = ctx.enter_context(tc.tile_pool(name="gram", bufs=1, space="PSUM"))
    pfin = ctx.enter_context(tc.tile_pool(name="fin", bufs=1, space="PSUM"))

    # small tables ride the sync queue ahead of the even feat stream; the
    # 1MB angular table takes the scalar queue ahead of the odd stream.
    oht = const.tile([128, NSC, 4, WCAP], _F32R)
    nc.sync.dma_start(oht[:], oh[:, :, :, :].bitcast(_F32R))
    wtt = const.tile([WCAP, NSC, D], _BF16)
    nc.sync.dma_start(wtt[:], wtb[:, :, :])
    cst = const.tile([128, 4, CS], _FP8A)
    nc.scalar.dma_start(cst[:], csT[:, :, :])
    ctn = const.tile([128, 4, C], _FP8A)
    nc.scalar.dma_start(ctn[:], ctnT[:, :, :])

    ones = const.tile([128, 1], _F32)
    nc.vector.memset(ones[:], 1.0)
    vacc = [
        const.tile([WCAP, D], _F32, name=f"vacc{k}", tag=f"vacc{k}")
        for k in range(2)
    ]
    nc.vector.memset(vacc[0][:], 0.0)
    ctb = const.tile([128, 1], _F32)
    nc.vector.memset(ctb[:], -CT)
    staging = const.tile([128, NSTG], _F32)
    nc.vector.memset(staging[:], 0.0)

    def supertile(st_i):
        # 2MB tile = 1024 rows = two 512-row window groups
        ft = pfeat.tile([128, 8, D], _F32R, tag="ft")
        eng = nc.sync if st_i % 2 == 0 else nc.scalar
        eng.dma_start(
            ft[:],
            feat[ds(st_i * 1024, 1024), :]
            .rearrange("(p s) d -> p s d", p=128)
            .bitcast(_F32R),
        )
        for gh in range(2):
            g = st_i * 2 + gh  # 512-row window group index
            # S[l, :] = sum of this group's feat rows with local class l
            S = pS.tile([WCAP, D], _F32, tag="S")
            for s in range(4):
                nc.tensor.matmul(
                    S[:],
                    oht[:, g, s, :],
                    ft[:, gh * 4 + s, :],
                    start=(s == 0),
                    stop=(s == 3),
                )
            # cross partial: V_acc += S * window; reduced once at the end
            tmp = pdump.tile([WCAP, D], _F32, tag="tmp")
            nc.vector.tensor_tensor(
                out=tmp[:], in0=S[:], in1=wtt[:, g, :], op=mybir.AluOpType.mult
            )
            nc.vector.tensor_tensor(
                out=vacc[(g + 1) % 2][:],
                in0=vacc[g % 2][:],
                in1=tmp[:],
                op=mybir.AluOpType.add,
            )
        # sum(f^2) partial for this tile
        sq = pdump.tile([128, 8, D], _BF16, tag="sq")
        nc.scalar.activation(
            sq[:],
            ft[:].bitcast(_F32),
            AF.Square,
            accum_out=staging[:, st_i : st_i + 1],
        )

    def angular_half(h):
        pg = pgram.tile([CS, 500], _F32, tag="pg")
        for ki in range(4):
            nc.tensor.matmul(
                pg[:],
                cst[:, ki, :],
                ctn[:, ki, ds(500 * h, 500)],
                start=(ki == 0),
                stop=(ki == 3),
            )
        gs = pang.tile([CS, 500], _F32, tag="gs")
        nc.scalar.activation(
            gs[:],
            pg[:],
            AF.Square,
            bias=ctb[:CS, :],
            accum_out=staging[:CS, 17 + h : 18 + h],
        )

    def halftile(ht_i, col):
        ft = pfeat.tile([128, 4, D], _F32R, tag="fth")
        eng = nc.sync if ht_i % 2 == 0 else nc.scalar
        eng.dma_start(
            ft[:],
            feat[ds(ht_i * 512, 512), :]
            .rearrange("(p s) d -> p s d", p=128)
            .bitcast(_F32R),
        )
        g = ht_i
        S = pS.tile([WCAP, D], _F32, tag="Sh")
        for s in range(4):
            nc.tensor.matmul(
                S[:],
                oht[:, g, s, :],
                ft[:, s, :],
                start=(s == 0),
                stop=(s == 3),
            )
        tmp = pdump.tile([WCAP, D], _F32, tag="tmph")
        nc.vector.tensor_tensor(
            out=tmp[:], in0=S[:], in1=wtt[:, g, :], op=mybir.AluOpType.mult
        )
        nc.vector.tensor_tensor(
            out=vacc[(g + 1) % 2][:],
            in0=vacc[g % 2][:],
            in1=tmp[:],
            op=mybir.AluOpType.add,
        )
        sq = pdump.tile([128, 4, D], _BF16, tag="sqh")
        nc.scalar.activation(
            sq[:],
            ft[:].bitcast(_F32),
            AF.Square,
            accum_out=staging[:, col : col + 1],
        )

    for st_i in range(8):
        supertile(st_i)
        if st_i == 4:
            angular_half(0)
        elif st_i == 6:
            angular_half(1)

    # collapse the cross accumulator: free-axis sum -> staging col 16
    vdmp = pdump.tile([WCAP, D], _BF16, tag="vdmp")
    nc.scalar.activation(
        vdmp[:],
        vacc[NSC % 2][:],
        AF.Copy,
        accum_out=staging[:WCAP, 16:17],
    )
    pf = pfin.tile([1, NSTG], _F32, tag="fin")
    nc.tensor.matmul(pf[:], ones[:], staging[:], start=True, stop=True)
    osb = const.tile([1, NSTG], _F32)
    nc.vector.tensor_copy(osb[:], pf[:])
    nc.sync.dma_start(out[:, :], osb[:, :])


def build():
    if "nc" in _NC_CACHE:
        return _NC_CACHE["nc"]
    nc = bacc.Bacc(
        "TRN2",
        target_bir_lowering=False,
        debug=False,
        enable_asserts=False,
        num_devices=N_CORES,
    )
    feat = nc.dram_tensor("feat", [BS, D], _F32, kind="ExternalInput").ap()
    oh = nc.dram_tensor("oh", [128, NSC, 4, WCAP], _F32, kind="ExternalInput").ap()
    wtb = nc.dram_tensor("wtb", [WCAP, NSC, D], _BF16, kind="ExternalInput").ap()
    ctnT = nc.dram_tensor("ctnT", [128, 4, C], _FP8A, kind="ExternalInput").ap()
    csT = nc.dram_tensor("csT", [128, 4, CS], _FP8A, kind="ExternalInput").ap()
    out = nc.dram_tensor("out", [1, NSTG], _F32, kind="ExternalOutput").ap()
    with tile.TileContext(nc) as tc, ExitStack() as ctx:
        _build_body(ctx, tc, feat, oh, wtb, ctnT, csT, out)
    nc.compile()
    _NC_CACHE["nc"] = nc
    return nc


def _dmajor(a):
    """[R, 512] -> [128, 4, R]: out[p, k, r] = a[r, 128*k + p]."""
    r = a.shape[0]
    return np.ascontiguousarray(a.T.reshape(4, 128, r).transpose(1, 0, 2))


def make_in_maps(y, feat, centers):
    """Returns (in_maps, cy2) where cy2 = sum_b ||c_{y_b}||^2 in f64."""
    feat = np.asarray(feat, dtype=np.float32)
    centers = np.asarray(centers, dtype=np.float32)
    y = np.asarray(y)

    order = np.argsort(y, kind="stable")
    ys = np.asarray(y)[order].astype(np.int32)

    c64 = centers.astype(np.float64)
    nsq = (c64**2).sum(1)  # ||c||^2 per class, f64
    counts = np.bincount(np.asarray(y).astype(np.int64), minlength=C)
    cy2 = float((counts * nsq).sum())

    norms = np.sqrt(nsq).astype(np.float32)
    ctn = (centers / norms[:, None]).astype(ml_dtypes.float8_e3m4)
    ctnT = _dmajor(ctn)  # [128, 4, 1000]
    cbf = centers.astype(ml_dtypes.bfloat16)

    in_maps = []
    for i in range(N_CORES):
        sl = slice(i * BS, (i + 1) * BS)
        fs = np.ascontiguousarray(feat[order[sl]])
        ysc = ys[sl]
        c0 = ysc.reshape(NSC, 512)[:, 0]  # sorted -> min class per superchunk
        lid = ysc.reshape(NSC, 128, 4) - c0[:, None, None]  # [sc, p, s]
        if lid.max() >= WCAP:
            raise ValueError(f"window span {lid.max() + 1} exceeds WCAP={WCAP}")
        oh = (lid[None, :, :, :] == np.arange(WCAP)[:, None, None, None]).astype(
            np.float32
        )  # [l, sc, p, s]
        oh = np.ascontiguousarray(oh.transpose(2, 1, 3, 0))  # [p, sc, s, l]
        rows = np.minimum(c0[None, :] + np.arange(WCAP)[:, None], C - 1)  # [l, sc]
        wtb = np.ascontiguousarray(cbf[rows])  # [WCAP, NSC, 512]
        csT = _dmajor(np.asarray(ctn[i * CS : (i + 1) * CS]))  # [128, 4, 125]
        in_maps.append(
            {"feat": fs, "oh": oh, "wtb": wtb, "ctnT": ctnT, "csT": csT}
        )
    return in_maps, cy2


def combine(outs, cy2):
    """outs: list of 8 [1,NSTG] f32 arrays + host cy2 -> scalar loss."""
    fsq = 0.0
    cross = 0.0
    ang = 0.0
    for o in outs:
        o = np.asarray(o, dtype=np.float64)
        fsq += o[0, 0:9].sum()
        cross += o[0, 16]
        ang += o[0, 17:19].sum()
    cen = fsq - 2.0 * cross + cy2
    ang -= C * (1.0 - CT) ** 2  # remove the diagonal (sim_ii == 1) terms
    loss = 0.5 * cen / B + ang / (0.5 * C * (C - 1))
    return np.float32(loss)


def kernel(y, feat, centers):
    nc = build()
    in_maps, cy2 = make_in_maps(y, feat, centers)
    res = run_bass_kernel_spmd(nc, in_maps, core_ids=list(range(N_CORES)))
    return combine([res.results[i]["out"] for i in range(N_CORES)], cy2)


# revision 18
# speedup vs baseline: 1.1314x; 1.0522x over previous
"""Trainium2 Bass kernel for AngelLoss (center loss + angular loss).

loss = 0.5*sum((feat - centers[y])^2)/B
     + sum_offdiag((c_i.c_j/(|c_i||c_j|) - ct)^2) / (0.5*C*(C-1))

Strategy (8 NeuronCores), sum-first decomposition:
  sum||f - c_y||^2 = sum||f||^2 - 2*sum f.c_y + sum||c_y||^2
  - The batch sum is order-invariant, so the host shards feat GLOBALLY
    SORTED by class (index prep only; all O(B*D) compute stays on
    device). Each 512-row superchunk then spans <= ~10 consecutive
    classes (cap 16).
  - sum||f||^2: square+accumulate straight off the feat stream
    (split ScalarE/DVE to balance engines).
  - sum f.c_y: per superchunk, S[l,:] = sum of feat rows with local
    class l via a one-hot matmul (16-column output, f32r moving rhs),
    then cross = <S, window> as one small DVE tensor_tensor_reduce.
  - sum||c_y||^2: exact host-side prep (class counts x center norms,
    O(B + C*D)) folded into combine().
  - Angular term: center rows sharded (125 rows/core); the host uploads
    the normalized center table pre-transposed (d-major), so the
    125x1000 Gram slice is 8 direct matmuls -- no on-device transposes.
  - per-core [1,40] partial sums are combined on the host.
"""

from contextlib import ExitStack

import ml_dtypes
import numpy as np

import concourse.bass as bass
import concourse.tile as tile
from concourse import bacc, mybir
from concourse.bass import ds, ts
from concourse.bass_utils import run_bass_kernel_spmd

N_CORES = 8
B, C, D = 65536, 1000, 512
BS = B // N_CORES  # 8192 rows per core
CS = C // N_CORES  # 125 gram rows per core
NSC = 16  # superchunks of 512 rows
WCAP = 16  # max classes spanned by one 512-row sorted superchunk (seed-0: 10)
NSTG = 20  # staging: 0..15 sum(f^2), 16 cross, 17..18 angular halves

# ct = 2*radius(C-1)^2 - 1 from the reference, evaluated in f64, cast f32.
CT = float(np.float32(-0.0010010010010047532))

_F32 = mybir.dt.float32
_F32R = mybir.dt.float32r
_BF16 = mybir.dt.bfloat16
_FP8A = mybir.dt.float8e3

_NC_CACHE = {}


def _build_body(ctx, tc, feat, oh, wtb, ctnT, csT, out):
    nc = tc.nc
    AF = mybir.ActivationFunctionType

    const = ctx.enter_context(tc.tile_pool(name="const", bufs=1))
    pfeat = ctx.enter_context(tc.tile_pool(name="feat", bufs=4))
    pdump = ctx.enter_context(tc.tile_pool(name="dump", bufs=2))
    pang = ctx.enter_context(tc.tile_pool(name="ang", bufs=2))
    pS = ctx.enter_context(tc.tile_pool(name="S", bufs=3, space="PSUM"))
    pgram = ctx.enter_context(tc.tile_pool(name="gram", bufs=1, space="PSUM"))
    pfin = ctx.enter_context(tc.tile_pool(name="fin", bufs=1, space="PSUM"))

    # small tables ride the sync queue ahead of the even feat stream; the
    # 1MB angular table takes the scalar queue ahead of the odd stream.
    oht = const.tile([128, NSC, 4, WCAP], _F32R)
    nc.sync.dma_start(oht[:], oh[:, :, :, :].bitcast(_F32R))
    wtt = const.tile([WCAP, NSC, D], _BF16)
    nc.sync.dma_start(wtt[:], wtb[:, :, :])
    cst = const.tile([128, 4, CS], _FP8A)
    nc.scalar.dma_start(cst[:], csT[:, :, :])
    ctn = const.tile([128, 4, C], _FP8A)
    nc.scalar.dma_start(ctn[:], ctnT[:, :, :])

    ones = const.tile([128, 1], _F32)
    nc.vector.memset(ones[:], 1.0)
    vacc = [
        const.tile([WCAP, D], _F32, name=f"vacc{k}", tag=f"vacc{k}")
        for k in range(2)
    ]
    nc.vector.memset(vacc[0][:], 0.0)
    ctb = const.tile([128, 1], _F32)
    nc.vector.memset(ctb[:], -CT)
    staging = const.tile([128, NSTG], _F32)
    nc.vector.memset(staging[:], 0.0)

    def supertile(st_i):
        # 2MB tile = 1024 rows = two 512-row window groups
        ft = pfeat.tile([128, 8, D], _F32R, tag="ft")
        eng = nc.sync if st_i % 2 == 0 else nc.scalar
        eng.dma_start(
            ft[:],
            feat[ds(st_i * 1024, 1024), :]
            .rearrange("(p s) d -> p s d", p=128)
            .bitcast(_F32R),
        )
        for gh in range(2):
            g = st_i * 2 + gh  # 512-row window group index
            # S[l, :] = sum of this group's feat rows with local class l
            S = pS.tile([WCAP, D], _F32, tag="S")
            for s in range(4):
                nc.tensor.matmul(
                    S[:],
                    oht[:, g, s, :],
                    ft[:, gh * 4 + s, :],
                    start=(s == 0),
                    stop=(s == 3),
                )
            # cross partial: V_acc += S * window; reduced once at the end
            tmp = pdump.tile([WCAP, D], _F32, tag="tmp")
            nc.vector.tensor_tensor(
                out=tmp[:], in0=S[:], in1=wtt[:, g, :], op=mybir.AluOpType.mult
            )
            nc.vector.tensor_tensor(
                out=vacc[(g + 1) % 2][:],
                in0=vacc[g % 2][:],
                in1=tmp[:],
                op=mybir.AluOpType.add,
            )
        # sum(f^2) partial for this tile
        sq = pdump.tile([128, 8, D], _BF16, tag="sq")
        nc.scalar.activation(
            sq[:],
            ft[:].bitcast(_F32),
            AF.Square,
            accum_out=staging[:, st_i : st_i + 1],
        )

    def angular_half(h):
        pg = pgram.tile([CS, 500], _F32, tag="pg")
        for ki in range(4):
            nc.tensor.matmul(
                pg[:],
                cst[:, ki, :],
                ctn[:, ki, ds(500 * h, 500)],
                start=(ki == 0),
                stop=(ki == 3),
            )
        g1 = pang.tile([CS, 500], _F32, tag="g1")
        nc.vector.tensor_scalar_add(g1[:], pg[:], ctb[:CS, :])
        g2 = pang.tile([CS, 500], _F32, tag="g2")
        nc.vector.tensor_tensor(
            out=g2[:], in0=g1[:], in1=g1[:], op=mybir.AluOpType.mult
        )
        nc.vector.tensor_reduce(
            out=staging[:CS, 17 + h : 18 + h],
            in_=g2[:],
            axis=mybir.AxisListType.X,
            op=mybir.AluOpType.add,
        )

    for st_i in range(8):
        supertile(st_i)
        if st_i == 4:
            angular_half(0)
        elif st_i == 6:
            angular_half(1)

    # collapse the cross accumulator: free-axis sum -> staging col 16
    nc.vector.tensor_reduce(
        out=staging[:WCAP, 16:17],
        in_=vacc[NSC % 2][:],
        axis=mybir.AxisListType.X,
        op=mybir.AluOpType.add,
    )
    pf = pfin.tile([1, NSTG], _F32, tag="fin")
    nc.tensor.matmul(pf[:], ones[:], staging[:], start=True, stop=True)
    osb = const.tile([1, NSTG], _F32)
    nc.vector.tensor_copy(osb[:], pf[:])
    nc.sync.dma_start(out[:, :], osb[:, :])


def build():
    if "nc" in _NC_CACHE:
        return _NC_CACHE["nc"]
    nc = bacc.Bacc(
        "TRN2",
        target_bir_lowering=False,
        debug=False,
        enable_asserts=False,
        num_devices=N_CORES,
    )
    feat = nc.dram_tensor("feat", [BS, D], _F32, kind="ExternalInput").ap()
    oh = nc.dram_tensor("oh", [128, NSC, 4, WCAP], _F32, kind="ExternalInput").ap()
    wtb = nc.dram_tensor("wtb", [WCAP, NSC, D], _BF16, kind="ExternalInput").ap()
    ctnT = nc.dram_tensor("ctnT", [128, 4, C], _FP8A, kind="ExternalInput").ap()
    csT = nc.dram_tensor("csT", [128, 4, CS], _FP8A, kind="ExternalInput").ap()
    out = nc.dram_tensor("out", [1, NSTG], _F32, kind="ExternalOutput").ap()
    with tile.TileContext(nc) as tc, ExitStack() as ctx:
        _build_body(ctx, tc, feat, oh, wtb, ctnT, csT, out)
    nc.compile()
    _NC_CACHE["nc"] = nc
    return nc


def _dmajor(a):
    """[R, 512] -> [128, 4, R]: out[p, k, r] = a[r, 128*k + p]."""
    r = a.shape[0]
    return np.ascontiguousarray(a.T.reshape(4, 128, r).transpose(1, 0, 2))


def make_in_maps(y, feat, centers):
    """Returns (in_maps, cy2) where cy2 = sum_b ||c_{y_b}||^2 in f64."""
    feat = np.asarray(feat, dtype=np.float32)
    centers = np.asarray(centers, dtype=np.float32)
    y = np.asarray(y)

    order = np.argsort(y, kind="stable")
    ys = np.asarray(y)[order].astype(np.int32)

    c64 = centers.astype(np.float64)
    nsq = (c64**2).sum(1)  # ||c||^2 per class, f64
    counts = np.bincount(np.asarray(y).astype(np.int64), minlength=C)
    cy2 = float((counts * nsq).sum())

    norms = np.sqrt(nsq).astype(np.float32)
    ctn = (centers / norms[:, None]).astype(ml_dtypes.float8_e3m4)
    ctnT = _dmajor(ctn)  # [128, 4, 1000]
    cbf = centers.astype(ml_dtypes.bfloat16)

    in_maps = []
    for i in range(N_CORES):
        sl = slice(i * BS, (i + 1) * BS)
        fs = np.ascontiguousarray(feat[order[sl]])
        ysc = ys[sl]
        c0 = ysc.reshape(NSC, 512)[:, 0]  # sorted -> min class per superchunk
        lid = ysc.reshape(NSC, 128, 4) - c0[:, None, None]  # [sc, p, s]
        if lid.max() >= WCAP:
            raise ValueError(f"window span {lid.max() + 1} exceeds WCAP={WCAP}")
        oh = (lid[None, :, :, :] == np.arange(WCAP)[:, None, None, None]).astype(
            np.float32
        )  # [l, sc, p, s]
        oh = np.ascontiguousarray(oh.transpose(2, 1, 3, 0))  # [p, sc, s, l]
        rows = np.minimum(c0[None, :] + np.arange(WCAP)[:, None], C - 1)  # [l, sc]
        wtb = np.ascontiguousarray(cbf[rows])  # [WCAP, NSC, 512]
        csT = _dmajor(np.asarray(ctn[i * CS : (i + 1) * CS]))  # [128, 4, 125]
        in_maps.append(
            {"feat": fs, "oh": oh, "wtb": wtb, "ctnT": ctnT, "csT": csT}
        )
    return in_maps, cy2


def combine(outs, cy2):
    """outs: list of 8 [1,NSTG] f32 arrays + host cy2 -> scalar loss."""
    fsq = 0.0
    cross = 0.0
    ang = 0.0
    for o in outs:
        o = np.asarray(o, dtype=np.float64)
        fsq += o[0, 0:9].sum()
        cross += o[0, 16]
        ang += o[0, 17:19].sum()
    cen = fsq - 2.0 * cross + cy2
    ang -= C * (1.0 - CT) ** 2  # remove the diagonal (sim_ii == 1) terms
    loss = 0.5 * cen / B + ang / (0.5 * C * (C - 1))
    return np.float32(loss)


def kernel(y, feat, centers):
    nc = build()
    in_maps, cy2 = make_in_maps(y, feat, centers)
    res = run_bass_kernel_spmd(nc, in_maps, core_ids=list(range(N_CORES)))
    return combine([res.results[i]["out"] for i in range(N_CORES)], cy2)
